# revision 13
# baseline (speedup 1.0000x reference)
"""BERT self-attention (B=2, S=2048, D=768, H=12) on 8 trn2 NeuronCores.

Sharding: core c -> batch b = c//4, head group g = c%4 (3 heads each).
Attention is fully local per core; no collectives.

v4 (all f32 storage; matmuls in float32r fast mode):
  - batched DMA (one dispatch per si-block of x^T / per weight tensor)
  - host folds log2(e)/8 into Wq so softmax exp becomes 2^scores:
    ACT path computes exp(ln2 * s), GPSIMD path computes pow(2, s) from an
    SBUF copy -- the exp work is split ACT 4/6, GPSIMD 2/6 per block
  - psB packing: one matmul produces [k_h2 | q_h2]; q_h2 lands via a
    partition-base-shifted DVE copy
  - P0: k+kq projections (all si), v (all t), q_h01(si0); P1 si-major
    attention with q-projections and epilogue transposes drained one item
    per exp-group into PE slack
"""

import sys

import numpy as np

_TRN_REPO = "/opt/trn_rl_repo"
if _TRN_REPO not in sys.path:
    sys.path.insert(0, _TRN_REPO)

import concourse.tile as tile  # noqa: E402
from concourse import bacc, mybir  # noqa: E402
from concourse.bass_utils import run_bass_kernel_spmd  # noqa: E402

F32 = mybir.dt.float32
F32R = mybir.dt.float32r
AF = mybir.ActivationFunctionType
OP = mybir.AluOpType

B, S, D = 2, 2048, 768
H_TOT, W = 12, 64
N_CORES = 8
HL = 3                # heads per core
DH = HL * W           # 192 local output dims
KC = D // 128         # 6 contraction chunks of 128
ST = 512              # s-tile (matmul moving free dim)
NS = S // ST          # 4 s-tiles
NT = S // 128         # 16 t-blocks
VPAD = 256            # v-projection free dim padded so float32r runs 1 cyc/row
LN2 = 0.6931471805599453
QSCALE = 0.1803368801111204  # log2(e)/8, folded into Wq on the host


def _round_f32r(a):
    """Round-to-nearest-even fp32 -> fp32r (11-bit mantissa, value kept in
    the top 20 bits of the word) so DMA'd data is already fp32r-valid."""
    u = np.ascontiguousarray(a, np.float32).view(np.uint32).copy()
    u += np.uint32(0x7FF) + ((u >> np.uint32(12)) & np.uint32(1))
    u &= np.uint32(0xFFFFF000)
    return u.view(np.float32)


def _emit(tc, aps, has_bias, has_mask):
    nc = tc.nc
    xt_d, wq_d, wk_d, wv_d, id_d, on_d, mb_d, out_d = aps

    from contextlib import ExitStack

    if has_mask:
        groups = [(t,) for t in range(NT)]      # bias varies per t-block
        pool_groups = set()                     # mask path: ACT only
    else:
        groups = [(0, 1, 2), (3, 4, 5), (6, 7, 8), (9, 10, 11),
                  (12, 13, 14), (15,)]
        pool_groups = {1, 3}                    # these go GPSIMD 2^x
    SCW = len(groups[0]) * ST

    with ExitStack() as ctx:
        const = ctx.enter_context(tc.tile_pool(name="const", bufs=1))

        ident = const.tile([128, 128], F32, name="ident", tag="ident")
        ones = const.tile([1, ST], F32R, name="ones", tag="ones")
        mb = None
        if has_mask:
            mb = const.tile([128, NT], F32, name="mb", tag="mb")
        two = None
        if pool_groups:
            two = const.tile([128, SCW], F32, name="two", tag="two")
            nc.gpsimd.memset(two[:], 2.0)

        # x^T as a single tile [128, KC, S]; weights one tile each.
        # wkall columns: 0:128 = k heads 0,1; 128:192 = k_h2; 192:256 = q_h2.
        xtall = const.tile([128, KC, S], F32R, name="xtall", tag="xtall")
        wkall = const.tile([128, KC, VPAD], F32R, name="wkall", tag="wkall")
        wqall = const.tile([128, KC, 128], F32R, name="wqall", tag="wqall")
        wvall = const.tile([128, KC, VPAD], F32R, name="wvall", tag="wvall")
        vaug = const.tile([128, NT, HL, W + 1], F32R, name="vaug", tag="vaug")
        bias_rows = const.tile([1, 3, VPAD], F32R, name="brows", tag="brows")

        xt = [xtall[:, c, :] for c in range(KC)]
        wk = [wkall[:, c, :] for c in range(KC)]
        wq = [wqall[:, c, :] for c in range(KC)]
        wv = [wvall[:, c, :] for c in range(KC)]
        wqb = bias_rows[:, 0, 0:128]
        wkb = bias_rows[:, 1, :]
        wvb = bias_rows[:, 2, :]

        xt_r = xt_d.rearrange("(c p) s -> p c s", p=128)
        wk_r = wk_d[0:D, :].rearrange("(c p) w -> p c w", p=128)
        wq_r = wq_d[0:D, :].rearrange("(c p) w -> p c w", p=128)
        wv_r = wv_d[0:D, :].rearrange("(c p) w -> p c w", p=128)

        # Batched DMA schedule: ~12 dispatches, ordered so k-proj starts
        # after the first two (wk + xt si0).
        nc.scalar.dma_start(out=wkall[:], in_=wk_r)
        nc.sync.dma_start(out=xtall[:, :, 0:ST], in_=xt_r[:, :, 0:ST])
        nc.gpsimd.dma_start(out=xtall[:, :, ST:2 * ST],
                            in_=xt_r[:, :, ST:2 * ST])
        nc.scalar.dma_start(out=wvall[:], in_=wv_r)
        nc.sync.dma_start(out=xtall[:, :, 2 * ST:3 * ST],
                          in_=xt_r[:, :, 2 * ST:3 * ST])
        nc.gpsimd.dma_start(out=xtall[:, :, 3 * ST:4 * ST],
                            in_=xt_r[:, :, 3 * ST:4 * ST])
        nc.scalar.dma_start(out=wqall[:], in_=wq_r)
        nc.sync.dma_start(
            out=vaug[:, :, :, W:W + 1],
            in_=on_d[0:128, 0:NT * HL].rearrange("p (t h b) -> p t h b",
                                                 h=HL, b=1))
        nc.gpsimd.dma_start(out=ident[:], in_=id_d[:, :])
        nc.sync.dma_start(out=ones[:], in_=on_d[0:1, :])
        if has_bias:
            nc.sync.dma_start(out=bias_rows[:, 0, 0:128],
                              in_=wq_d[D:D + 1, :])
            nc.sync.dma_start(out=bias_rows[:, 1, :], in_=wk_d[D:D + 1, :])
            nc.sync.dma_start(out=bias_rows[:, 2, :], in_=wv_d[D:D + 1, :])
        if has_mask:
            nc.gpsimd.dma_start(out=mb[:], in_=mb_d[:, :])

        # Projection outputs (persistent). q/k zero-padded on complementary
        # 64 partitions so scores matmuls run K=128.
        qt_h = [const.tile([128, S], F32R, name=f"qt_h{h}", tag=f"qt_h{h}")
                for h in range(HL)]
        kt_a = const.tile([128, S], F32R, name="kt_a", tag="kt_a")
        kt_b = const.tile([128, S], F32R, name="kt_b", tag="kt_b")
        # Zero the complementary K-padding halves on DVE: in*0.0 from an
        # already-loaded (finite) x column block, piecewise per si so each
        # piece only depends on its own xt DMA. (memset rejects f32r.)
        for si in range(NS):
            ssl = slice(si * ST, (si + 1) * ST)
            nc.vector.tensor_scalar_mul(qt_h[0][64:128, ssl],
                                        xt[0][0:64, ssl], 0.0)
            nc.vector.tensor_scalar_mul(qt_h[1][0:64, ssl],
                                        xt[0][0:64, ssl], 0.0)
            nc.vector.tensor_scalar_mul(qt_h[2][64:128, ssl],
                                        xt[0][0:64, ssl], 0.0)
            nc.vector.tensor_scalar_mul(kt_b[64:128, ssl],
                                        xt[0][0:64, ssl], 0.0)

        # ---- P0 ---------------------------------------------------------
        def kproj_mms(si, psA, psB):
            ssl = slice(si * ST, (si + 1) * ST)
            for c in range(KC):
                nc.tensor.matmul(
                    psA[:], wk[c][:, 0:128], xt[c][:, ssl],
                    start=(c == 0), stop=(c == KC - 1 and not has_bias))
            if has_bias:
                nc.tensor.matmul(psA[:], wkb[:, 0:128], ones[:],
                                 start=False, stop=True)
            for c in range(KC):
                nc.tensor.matmul(
                    psB[:], wk[c][:, 128:VPAD], xt[c][:, ssl],
                    start=(c == 0), stop=(c == KC - 1 and not has_bias))
            if has_bias:
                nc.tensor.matmul(psB[:], wkb[:, 128:VPAD], ones[:],
                                 start=False, stop=True)

        def kproj_copies(si, psA, psB):
            ssl = slice(si * ST, (si + 1) * ST)
            nc.vector.tensor_copy(kt_a[:, ssl], psA[:])
            nc.vector.tensor_copy(kt_b[0:64, ssl], psB[0:64, :])
            # q_h2 rides in psB rows 64:128 -> partition-shifted copy
            nc.vector.tensor_copy(qt_h[2][0:64, ssl], psB[64:128, :])

        def qproj_mms(si, psA):
            ssl = slice(si * ST, (si + 1) * ST)
            for c in range(KC):
                nc.tensor.matmul(
                    psA[:], wq[c][:], xt[c][:, ssl],
                    start=(c == 0), stop=(c == KC - 1 and not has_bias))
            if has_bias:
                nc.tensor.matmul(psA[:], wqb[:], ones[:],
                                 start=False, stop=True)

        def qproj_copies(si, psA):
            ssl = slice(si * ST, (si + 1) * ST)
            nc.vector.tensor_copy(qt_h[0][0:64, ssl], psA[0:64, :])
            nc.vector.tensor_copy(qt_h[1][64:128, ssl], psA[64:128, :])

        with tc.tile_pool(name="pA", bufs=2, space="PSUM") as pA, \
             tc.tile_pool(name="pB", bufs=2, space="PSUM") as pB, \
             tc.tile_pool(name="pV", bufs=2, space="PSUM") as pV:

            for si in range(NS):
                psA = pA.tile([128, ST], F32, name="psA", tag="psA")
                psB = pB.tile([128, ST], F32, name="psB", tag="psB")
                kproj_mms(si, psA, psB)
                kproj_copies(si, psA, psB)
            for t in range(NT):
                psV = pV.tile([128, VPAD], F32, name="psV", tag="psV")
                tsl = slice(t * 128, (t + 1) * 128)
                for c in range(KC):
                    nc.tensor.matmul(
                        psV[:], xt[c][:, tsl], wv[c][:],
                        start=(c == 0), stop=(c == KC - 1 and not has_bias))
                if has_bias:
                    nc.tensor.matmul(psV[:], ones[:, 0:128], wvb[:],
                                     start=False, stop=True)
                nc.vector.tensor_copy(
                    vaug[:, t, :, 0:W],
                    psV[:, 0:DH].rearrange("p (h w) -> p h w", h=HL))
            psA = pA.tile([128, ST], F32, name="psA", tag="psA")
            qproj_mms(0, psA)
            qproj_copies(0, psA)

        # ---- P1: attention, si-major ------------------------------------
        with tc.tile_pool(name="scps", bufs=2, space="PSUM") as scp, \
             tc.tile_pool(name="ctxps", bufs=1, space="PSUM") as cxp, \
             tc.tile_pool(name="expool", bufs=3) as exp_pool, \
             tc.tile_pool(name="scsb", bufs=2) as scsb, \
             tc.tile_pool(name="ctxsb", bufs=3) as csb, \
             tc.tile_pool(name="outsb", bufs=3) as osb:

            from contextlib import ExitStack as _ES
            late = _ES()
            trp = None            # transpose pool, opened late
            work = []             # FIFO: q-proj items, then epilogue items

            qx_pool = tc.tile_pool(name="qx", bufs=1, space="PSUM")
            qx = qx_pool.__enter__()
            qx_closed = [False]

            def close_qx():
                nonlocal trp
                if qx_closed[0]:
                    return
                qx_closed[0] = True
                qx_pool.__exit__(None, None, None)
                trp = late.enter_context(
                    tc.tile_pool(name="trps", bufs=1, space="PSUM"))

            def drain_work(n):
                for _ in range(n):
                    if work:
                        work.pop(0)()

            def make_qproj(si):
                items = []
                state = {}
                ssl = slice(si * ST, (si + 1) * ST)

                def alloc():
                    state["t"] = qx.tile([128, ST], F32, name="qxp",
                                         tag="qxp")
                items.append(alloc)
                for c in range(KC):
                    def mmA(c=c):
                        nc.tensor.matmul(
                            state["t"][:], wq[c][:], xt[c][:, ssl],
                            start=(c == 0),
                            stop=(c == KC - 1 and not has_bias))
                    items.append(mmA)
                if has_bias:
                    items.append(lambda: nc.tensor.matmul(
                        state["t"][:], wqb[:], ones[:],
                        start=False, stop=True))

                def cpA():
                    nc.vector.tensor_copy(qt_h[0][0:64, ssl],
                                          state["t"][0:64, :])
                    nc.vector.tensor_copy(qt_h[1][64:128, ssl],
                                          state["t"][64:128, :])
                items.append(cpA)
                return items

            def make_epilogue(h, si, ctx_ps):
                # copy PSUM ctx out immediately (on ACT, which has slack);
                # transposes become queue items drained into PE slack
                ctx_sb = csb.tile([65, ST], F32, name="ctx_sb",
                                  tag="ctx_sb")
                nc.scalar.copy(ctx_sb[:], ctx_ps[:])
                items = []
                for j in range(NS):
                    def tr_item(j=j):
                        close_qx()
                        jsl = slice(j * 128, (j + 1) * 128)
                        tr_ps = trp.tile([128, 65], F32, name="tr_ps",
                                         tag="tr_ps")
                        nc.tensor.transpose(
                            tr_ps[:], ctx_sb[:, jsl], ident[0:65, 0:65])
                        rec = osb.tile([128, 1], F32, name="rec", tag="rec")
                        nc.vector.reciprocal(rec[:], tr_ps[:, W:W + 1])
                        outt = osb.tile([128, W], F32, name="outt",
                                        tag="outt")
                        nc.vector.tensor_scalar_mul(outt[:], tr_ps[:, 0:W],
                                                    rec[:])
                        nc.sync.dma_start(
                            out=out_d[si * ST + j * 128:
                                      si * ST + (j + 1) * 128,
                                      h * W:(h + 1) * W],
                            in_=outt[:])
                    items.append(tr_item)
                return items

            for si in range(1, NS):
                work.extend(make_qproj(si))

            for si in range(NS):
                for h in range(HL):
                    ktile = kt_a if h < 2 else kt_b
                    qtile = qt_h[h]
                    ssl = slice(si * ST, (si + 1) * ST)
                    ctx_ps = cxp.tile([65, ST], F32, name="ctx_ps",
                                      tag="ctx_ps")
                    prev = None
                    n_acc = [0]

                    def ctx_mms(ex, tlist, last):
                        for j, t in enumerate(tlist):
                            nc.tensor.matmul(
                                ctx_ps[:], vaug[:, t, h, :],
                                ex[:, j * ST:(j + 1) * ST],
                                start=(n_acc[0] == 0),
                                stop=(last and j == len(tlist) - 1))
                            n_acc[0] += 1

                    for gi, tlist in enumerate(groups):
                        gw = len(tlist) * ST
                        sc_ps = scp.tile([128, SCW], F32, name="sc_ps",
                                         tag="sc_ps")
                        for j, t in enumerate(tlist):
                            nc.tensor.matmul(
                                sc_ps[:, j * ST:(j + 1) * ST],
                                ktile[:, t * 128:(t + 1) * 128],
                                qtile[:, ssl], start=True, stop=True)
                        drain_work(2)
                        ex = exp_pool.tile([128, SCW], F32R, name="ex",
                                           tag="ex")
                        if gi in pool_groups:
                            scb = scsb.tile([128, SCW], F32, name="scb",
                                            tag="scb")
                            nc.vector.tensor_copy(scb[:, 0:gw],
                                                  sc_ps[:, 0:gw])
                            nc.gpsimd.tensor_tensor(
                                out=ex[:, 0:gw], in0=two[:, 0:gw],
                                in1=scb[:, 0:gw], op=OP.pow)
                        else:
                            nc.scalar.activation(
                                ex[:, 0:gw], sc_ps[:, 0:gw], AF.Exp,
                                bias=(mb[:, tlist[0]:tlist[0] + 1]
                                      if has_mask else 0.0),
                                scale=LN2)
                        if prev is not None:
                            ctx_mms(prev[0], prev[1], last=False)
                        prev = (ex, tlist)
                    ctx_mms(prev[0], prev[1], last=True)
                    work.extend(make_epilogue(h, si, ctx_ps))
            while work:
                work.pop(0)()
            close_qx()
            late.close()


def _build(has_bias, has_mask):
    nc = bacc.Bacc(
        "TRN2", target_bir_lowering=False, debug=False, num_devices=N_CORES
    )
    xt_d = nc.dram_tensor("xt", [D, S], F32R, kind="ExternalInput").ap()
    wq_d = nc.dram_tensor("wq", [D + 1, 128], F32R, kind="ExternalInput").ap()
    wk_d = nc.dram_tensor("wk", [D + 1, VPAD], F32R,
                          kind="ExternalInput").ap()
    wv_d = nc.dram_tensor("wv", [D + 1, VPAD], F32R,
                          kind="ExternalInput").ap()
    id_d = nc.dram_tensor("ident", [128, 128], F32,
                          kind="ExternalInput").ap()
    on_d = nc.dram_tensor("onesd", [128, ST], F32R, kind="ExternalInput").ap()
    mb_d = (
        nc.dram_tensor("mb", [128, NT], F32, kind="ExternalInput").ap()
        if has_mask else None
    )
    out_d = nc.dram_tensor("out", [S, DH], F32, kind="ExternalOutput").ap()

    with tile.TileContext(nc) as tc:
        _emit(tc, (xt_d, wq_d, wk_d, wv_d, id_d, on_d, mb_d, out_d),
              has_bias, has_mask)
    nc.compile()
    return nc


_NC_CACHE = {}


def _get_nc(has_bias, has_mask):
    key = (has_bias, has_mask)
    if key not in _NC_CACHE:
        _NC_CACHE[key] = _build(has_bias, has_mask)
    return _NC_CACHE[key]


def _in_maps(x, Wq, bq, Wk, bk, Wv, bv, mask, has_bias, has_mask):
    ident = np.eye(128, dtype=np.float32)
    xt_by_b = [np.ascontiguousarray(x[b].T) for b in range(B)]
    mb_by_b = [
        np.ascontiguousarray(
            ((np.asarray(mask[b]) == 0).astype(np.float32) * np.float32(-1e30))
            .reshape(NT, 128).T
        )
        for b in range(B)
    ]
    maps = []
    for c in range(N_CORES):
        b, g = divmod(c, N_CORES // B)
        lo = g * DH
        # wq: q heads 0,1 (128 douts), scaled by log2(e)/8
        wq_a = np.empty((D + 1, 128), np.float32)
        wq_a[:D] = Wq[lo:lo + 128, :].T * QSCALE
        wq_a[D] = bq[lo:lo + 128] * QSCALE
        # wk: [k_h01 (128) | k_h2 (64) | q_h2 (64, scaled)]
        wk_a = np.empty((D + 1, VPAD), np.float32)
        wk_a[:D, 0:128] = Wk[lo:lo + 128, :].T
        wk_a[D, 0:128] = bk[lo:lo + 128]
        wk_a[:D, 128:DH] = Wk[lo + 128:lo + DH, :].T
        wk_a[D, 128:DH] = bk[lo + 128:lo + DH]
        wk_a[:D, DH:VPAD] = Wq[lo + 128:lo + DH, :].T * QSCALE
        wk_a[D, DH:VPAD] = bq[lo + 128:lo + DH] * QSCALE
        wv_a = np.zeros((D + 1, VPAD), np.float32)
        wv_a[:D, :DH] = Wv[lo:lo + DH, :].T
        wv_a[D, :DH] = bv[lo:lo + DH]
        m = {
            "xt": _round_f32r(xt_by_b[b]), "wq": _round_f32r(wq_a),
            "wk": _round_f32r(wk_a), "wv": _round_f32r(wv_a), "ident": ident,
            "onesd": np.ones((128, ST), np.float32),
        }
        if has_mask:
            m["mb"] = mb_by_b[b]
        maps.append(m)
    return maps


def _install_ntff_hook():
    """Best-effort: make trace=True work under axon by supplying the
    antenv.axon_hooks shim the boot code degrades without."""
    import types

    try:
        from antenv.axon_hooks import get_axon_ntff_profile_hook  # noqa: F401
        return True
    except ImportError:
        pass
    try:
        import antenv
        from trn_agent_boot.trn_boot import _ntff_profile_via_ctypes

        hook = _ntff_profile_via_ctypes("/opt/axon/libaxon_pjrt.so")
        if hook is None:
            return False
        mod = types.ModuleType("antenv.axon_hooks")
        state = {"hook": hook}
        mod.get_axon_ntff_profile_hook = lambda: state["hook"]
        mod.set_axon_ntff_profile_hook = lambda h: state.update(hook=h)
        sys.modules["antenv.axon_hooks"] = mod
        antenv.axon_hooks = mod
        return True
    except Exception:
        return False


def _run(x, Wq, bq, Wk, bk, Wv, bv, mask, trace=False):
    if trace:
        trace = _install_ntff_hook()
    x = np.ascontiguousarray(np.asarray(x, np.float32))
    Wq = np.asarray(Wq, np.float32)
    Wk = np.asarray(Wk, np.float32)
    Wv = np.asarray(Wv, np.float32)
    bq = np.asarray(bq, np.float32)
    bk = np.asarray(bk, np.float32)
    bv = np.asarray(bv, np.float32)
    has_bias = bool(np.any(bq) or np.any(bk) or np.any(bv))
    has_mask = bool((np.asarray(mask) == 0).any())
    nc = _get_nc(has_bias, has_mask)
    maps = _in_maps(x, Wq, bq, Wk, bk, Wv, bv, mask, has_bias, has_mask)
    res = run_bass_kernel_spmd(nc, maps, list(range(N_CORES)), trace=trace)
    out = np.empty((B, S, D), np.float32)
    for c in range(N_CORES):
        b, g = divmod(c, N_CORES // B)
        out[b, :, g * DH:(g + 1) * DH] = res.results[c]["out"]
    return out, res


def kernel(x, Wq, bq, Wk, bk, Wv, bv, mask):
    out, _ = _run(x, Wq, bq, Wk, bk, Wv, bv, mask)
    return out


# revision 14
# speedup vs baseline: 37.8269x; 37.8269x over previous
"""BERT self-attention (B=2, S=2048, D=768, H=12) on 8 trn2 NeuronCores.

Sharding: core c -> batch b = c//4, head group g = c%4 (3 heads each).
Attention is fully local per core; no collectives.

v4 (all f32 storage; matmuls in float32r fast mode):
  - batched DMA (one dispatch per si-block of x^T / per weight tensor)
  - host folds log2(e)/8 into Wq so softmax exp becomes 2^scores:
    ACT path computes exp(ln2 * s), GPSIMD path computes pow(2, s) from an
    SBUF copy -- the exp work is split ACT 4/6, GPSIMD 2/6 per block
  - psB packing: one matmul produces [k_h2 | q_h2]; q_h2 lands via a
    partition-base-shifted DVE copy
  - P0: k+kq projections (all si), v (all t), q_h01(si0); P1 si-major
    attention with q-projections and epilogue transposes drained one item
    per exp-group into PE slack
"""

import sys

import numpy as np

_TRN_REPO = "/opt/trn_rl_repo"
if _TRN_REPO not in sys.path:
    sys.path.insert(0, _TRN_REPO)

import concourse.tile as tile  # noqa: E402
from concourse import bacc, mybir  # noqa: E402
from concourse.bass_utils import run_bass_kernel_spmd  # noqa: E402

F32 = mybir.dt.float32
F32R = mybir.dt.float32r
AF = mybir.ActivationFunctionType
OP = mybir.AluOpType

B, S, D = 2, 2048, 768
H_TOT, W = 12, 64
N_CORES = 8
HL = 3                # heads per core
DH = HL * W           # 192 local output dims
KC = D // 128         # 6 contraction chunks of 128
ST = 512              # s-tile (matmul moving free dim)
NS = S // ST          # 4 s-tiles
NT = S // 128         # 16 t-blocks
VPAD = 256            # v-projection free dim padded so float32r runs 1 cyc/row
LN2 = 0.6931471805599453
QSCALE = 0.1803368801111204  # log2(e)/8, folded into Wq on the host


def _round_f32r(a):
    """Round-to-nearest-even fp32 -> fp32r (11-bit mantissa, value kept in
    the top 20 bits of the word) so DMA'd data is already fp32r-valid."""
    u = np.ascontiguousarray(a, np.float32).view(np.uint32).copy()
    u += np.uint32(0x7FF) + ((u >> np.uint32(12)) & np.uint32(1))
    u &= np.uint32(0xFFFFF000)
    return u.view(np.float32)


def _emit(tc, aps, has_bias, has_mask):
    nc = tc.nc
    xt_d, wq_d, wk_d, wv_d, id_d, on_d, mb_d, out_d = aps

    from contextlib import ExitStack

    if has_mask:
        groups = [(t,) for t in range(NT)]      # bias varies per t-block
        pool_groups = set()                     # mask path: ACT only
    else:
        groups = [(0, 1, 2), (3, 4, 5), (6, 7, 8), (9, 10, 11),
                  (12, 13, 14), (15,)]
        pool_groups = set()   # GPSIMD pow measured ~300us/tile on HW: dead
    SCW = len(groups[0]) * ST

    with ExitStack() as ctx:
        const = ctx.enter_context(tc.tile_pool(name="const", bufs=1))

        ident = const.tile([128, 128], F32, name="ident", tag="ident")
        ones = const.tile([1, ST], F32R, name="ones", tag="ones")
        mb = None
        if has_mask:
            mb = const.tile([128, NT], F32, name="mb", tag="mb")
        two = None
        if pool_groups:
            two = const.tile([128, SCW], F32, name="two", tag="two")
            nc.gpsimd.memset(two[:], 2.0)

        # x^T as a single tile [128, KC, S]; weights one tile each.
        # wkall columns: 0:128 = k heads 0,1; 128:192 = k_h2; 192:256 = q_h2.
        xtall = const.tile([128, KC, S], F32R, name="xtall", tag="xtall")
        wkall = const.tile([128, KC, VPAD], F32R, name="wkall", tag="wkall")
        wqall = const.tile([128, KC, 128], F32R, name="wqall", tag="wqall")
        wvall = const.tile([128, KC, VPAD], F32R, name="wvall", tag="wvall")
        vaug = const.tile([128, NT, HL, W + 1], F32R, name="vaug", tag="vaug")
        bias_rows = const.tile([1, 3, VPAD], F32R, name="brows", tag="brows")

        xt = [xtall[:, c, :] for c in range(KC)]
        wk = [wkall[:, c, :] for c in range(KC)]
        wq = [wqall[:, c, :] for c in range(KC)]
        wv = [wvall[:, c, :] for c in range(KC)]
        wqb = bias_rows[:, 0, 0:128]
        wkb = bias_rows[:, 1, :]
        wvb = bias_rows[:, 2, :]

        xt_r = xt_d.rearrange("(c p) s -> p c s", p=128)
        wk_r = wk_d[0:D, :].rearrange("(c p) w -> p c w", p=128)
        wq_r = wq_d[0:D, :].rearrange("(c p) w -> p c w", p=128)
        wv_r = wv_d[0:D, :].rearrange("(c p) w -> p c w", p=128)

        # Batched DMA schedule: ~12 dispatches, ordered so k-proj starts
        # after the first two (wk + xt si0).
        nc.scalar.dma_start(out=wkall[:], in_=wk_r)
        nc.sync.dma_start(out=xtall[:, :, 0:ST], in_=xt_r[:, :, 0:ST])
        nc.gpsimd.dma_start(out=xtall[:, :, ST:2 * ST],
                            in_=xt_r[:, :, ST:2 * ST])
        nc.scalar.dma_start(out=wvall[:], in_=wv_r)
        nc.sync.dma_start(out=xtall[:, :, 2 * ST:3 * ST],
                          in_=xt_r[:, :, 2 * ST:3 * ST])
        nc.gpsimd.dma_start(out=xtall[:, :, 3 * ST:4 * ST],
                            in_=xt_r[:, :, 3 * ST:4 * ST])
        nc.scalar.dma_start(out=wqall[:], in_=wq_r)
        nc.sync.dma_start(
            out=vaug[:, :, :, W:W + 1],
            in_=on_d[0:128, 0:NT * HL].rearrange("p (t h b) -> p t h b",
                                                 h=HL, b=1))
        nc.gpsimd.dma_start(out=ident[:], in_=id_d[:, :])
        nc.sync.dma_start(out=ones[:], in_=on_d[0:1, :])
        if has_bias:
            nc.sync.dma_start(out=bias_rows[:, 0, 0:128],
                              in_=wq_d[D:D + 1, :])
            nc.sync.dma_start(out=bias_rows[:, 1, :], in_=wk_d[D:D + 1, :])
            nc.sync.dma_start(out=bias_rows[:, 2, :], in_=wv_d[D:D + 1, :])
        if has_mask:
            nc.gpsimd.dma_start(out=mb[:], in_=mb_d[:, :])

        # Projection outputs (persistent). q/k zero-padded on complementary
        # 64 partitions so scores matmuls run K=128.
        qt_h = [const.tile([128, S], F32R, name=f"qt_h{h}", tag=f"qt_h{h}")
                for h in range(HL)]
        kt_a = const.tile([128, S], F32R, name="kt_a", tag="kt_a")
        kt_b = const.tile([128, S], F32R, name="kt_b", tag="kt_b")
        # Zero the complementary K-padding halves on DVE: in*0.0 from an
        # already-loaded (finite) x column block, piecewise per si so each
        # piece only depends on its own xt DMA. (memset rejects f32r.)
        for si in range(NS):
            ssl = slice(si * ST, (si + 1) * ST)
            nc.vector.tensor_scalar_mul(qt_h[0][64:128, ssl],
                                        xt[0][0:64, ssl], 0.0)
            nc.vector.tensor_scalar_mul(qt_h[1][0:64, ssl],
                                        xt[0][0:64, ssl], 0.0)
            nc.vector.tensor_scalar_mul(qt_h[2][64:128, ssl],
                                        xt[0][0:64, ssl], 0.0)
            nc.vector.tensor_scalar_mul(kt_b[64:128, ssl],
                                        xt[0][0:64, ssl], 0.0)

        # ---- P0 ---------------------------------------------------------
        def kproj_mms(si, psA, psB):
            ssl = slice(si * ST, (si + 1) * ST)
            for c in range(KC):
                nc.tensor.matmul(
                    psA[:], wk[c][:, 0:128], xt[c][:, ssl],
                    start=(c == 0), stop=(c == KC - 1 and not has_bias))
            if has_bias:
                nc.tensor.matmul(psA[:], wkb[:, 0:128], ones[:],
                                 start=False, stop=True)
            for c in range(KC):
                nc.tensor.matmul(
                    psB[:], wk[c][:, 128:VPAD], xt[c][:, ssl],
                    start=(c == 0), stop=(c == KC - 1 and not has_bias))
            if has_bias:
                nc.tensor.matmul(psB[:], wkb[:, 128:VPAD], ones[:],
                                 start=False, stop=True)

        def kproj_copies(si, psA, psB):
            ssl = slice(si * ST, (si + 1) * ST)
            nc.vector.tensor_copy(kt_a[:, ssl], psA[:])
            nc.vector.tensor_copy(kt_b[0:64, ssl], psB[0:64, :])
            # q_h2 rides in psB rows 64:128 -> partition-shifted copy
            nc.vector.tensor_copy(qt_h[2][0:64, ssl], psB[64:128, :])

        def qproj_mms(si, psA):
            ssl = slice(si * ST, (si + 1) * ST)
            for c in range(KC):
                nc.tensor.matmul(
                    psA[:], wq[c][:], xt[c][:, ssl],
                    start=(c == 0), stop=(c == KC - 1 and not has_bias))
            if has_bias:
                nc.tensor.matmul(psA[:], wqb[:], ones[:],
                                 start=False, stop=True)

        def qproj_copies(si, psA):
            ssl = slice(si * ST, (si + 1) * ST)
            nc.vector.tensor_copy(qt_h[0][0:64, ssl], psA[0:64, :])
            nc.vector.tensor_copy(qt_h[1][64:128, ssl], psA[64:128, :])

        with tc.tile_pool(name="pA", bufs=2, space="PSUM") as pA, \
             tc.tile_pool(name="pB", bufs=2, space="PSUM") as pB, \
             tc.tile_pool(name="pV", bufs=2, space="PSUM") as pV:

            for si in range(NS):
                psA = pA.tile([128, ST], F32, name="psA", tag="psA")
                psB = pB.tile([128, ST], F32, name="psB", tag="psB")
                kproj_mms(si, psA, psB)
                kproj_copies(si, psA, psB)
            for t in range(NT):
                psV = pV.tile([128, VPAD], F32, name="psV", tag="psV")
                tsl = slice(t * 128, (t + 1) * 128)
                for c in range(KC):
                    nc.tensor.matmul(
                        psV[:], xt[c][:, tsl], wv[c][:],
                        start=(c == 0), stop=(c == KC - 1 and not has_bias))
                if has_bias:
                    nc.tensor.matmul(psV[:], ones[:, 0:128], wvb[:],
                                     start=False, stop=True)
                nc.vector.tensor_copy(
                    vaug[:, t, :, 0:W],
                    psV[:, 0:DH].rearrange("p (h w) -> p h w", h=HL))
            psA = pA.tile([128, ST], F32, name="psA", tag="psA")
            qproj_mms(0, psA)
            qproj_copies(0, psA)

        # ---- P1: attention, si-major ------------------------------------
        with tc.tile_pool(name="scps", bufs=2, space="PSUM") as scp, \
             tc.tile_pool(name="ctxps", bufs=1, space="PSUM") as cxp, \
             tc.tile_pool(name="expool", bufs=3) as exp_pool, \
             tc.tile_pool(name="scsb", bufs=2) as scsb, \
             tc.tile_pool(name="ctxsb", bufs=3) as csb, \
             tc.tile_pool(name="outsb", bufs=3) as osb:

            from contextlib import ExitStack as _ES
            late = _ES()
            trp = None            # transpose pool, opened late
            work = []             # FIFO: q-proj items, then epilogue items

            qx_pool = tc.tile_pool(name="qx", bufs=1, space="PSUM")
            qx = qx_pool.__enter__()
            qx_closed = [False]

            def close_qx():
                nonlocal trp
                if qx_closed[0]:
                    return
                qx_closed[0] = True
                qx_pool.__exit__(None, None, None)
                trp = late.enter_context(
                    tc.tile_pool(name="trps", bufs=1, space="PSUM"))

            def drain_work(n):
                for _ in range(n):
                    if work:
                        work.pop(0)()

            def make_qproj(si):
                items = []
                state = {}
                ssl = slice(si * ST, (si + 1) * ST)

                def alloc():
                    state["t"] = qx.tile([128, ST], F32, name="qxp",
                                         tag="qxp")
                items.append(alloc)
                for c in range(KC):
                    def mmA(c=c):
                        nc.tensor.matmul(
                            state["t"][:], wq[c][:], xt[c][:, ssl],
                            start=(c == 0),
                            stop=(c == KC - 1 and not has_bias))
                    items.append(mmA)
                if has_bias:
                    items.append(lambda: nc.tensor.matmul(
                        state["t"][:], wqb[:], ones[:],
                        start=False, stop=True))

                def cpA():
                    nc.vector.tensor_copy(qt_h[0][0:64, ssl],
                                          state["t"][0:64, :])
                    nc.vector.tensor_copy(qt_h[1][64:128, ssl],
                                          state["t"][64:128, :])
                items.append(cpA)
                return items

            def make_epilogue(h, si, ctx_ps):
                # copy PSUM ctx out immediately (on ACT, which has slack);
                # transposes become queue items drained into PE slack
                ctx_sb = csb.tile([65, ST], F32, name="ctx_sb",
                                  tag="ctx_sb")
                nc.scalar.copy(ctx_sb[:], ctx_ps[:])
                items = []
                for j in range(NS):
                    def tr_item(j=j):
                        close_qx()
                        jsl = slice(j * 128, (j + 1) * 128)
                        tr_ps = trp.tile([128, 65], F32, name="tr_ps",
                                         tag="tr_ps")
                        nc.tensor.transpose(
                            tr_ps[:], ctx_sb[:, jsl], ident[0:65, 0:65])
                        rec = osb.tile([128, 1], F32, name="rec", tag="rec")
                        nc.vector.reciprocal(rec[:], tr_ps[:, W:W + 1])
                        outt = osb.tile([128, W], F32, name="outt",
                                        tag="outt")
                        nc.vector.tensor_scalar_mul(outt[:], tr_ps[:, 0:W],
                                                    rec[:])
                        nc.sync.dma_start(
                            out=out_d[si * ST + j * 128:
                                      si * ST + (j + 1) * 128,
                                      h * W:(h + 1) * W],
                            in_=outt[:])
                    items.append(tr_item)
                return items

            for si in range(1, NS):
                work.extend(make_qproj(si))

            for si in range(NS):
                for h in range(HL):
                    ktile = kt_a if h < 2 else kt_b
                    qtile = qt_h[h]
                    ssl = slice(si * ST, (si + 1) * ST)
                    ctx_ps = cxp.tile([65, ST], F32, name="ctx_ps",
                                      tag="ctx_ps")
                    prev = None
                    n_acc = [0]

                    def ctx_mms(ex, tlist, last):
                        for j, t in enumerate(tlist):
                            nc.tensor.matmul(
                                ctx_ps[:], vaug[:, t, h, :],
                                ex[:, j * ST:(j + 1) * ST],
                                start=(n_acc[0] == 0),
                                stop=(last and j == len(tlist) - 1))
                            n_acc[0] += 1

                    for gi, tlist in enumerate(groups):
                        gw = len(tlist) * ST
                        sc_ps = scp.tile([128, SCW], F32, name="sc_ps",
                                         tag="sc_ps")
                        for j, t in enumerate(tlist):
                            nc.tensor.matmul(
                                sc_ps[:, j * ST:(j + 1) * ST],
                                ktile[:, t * 128:(t + 1) * 128],
                                qtile[:, ssl], start=True, stop=True)
                        drain_work(2)
                        ex = exp_pool.tile([128, SCW], F32R, name="ex",
                                           tag="ex")
                        if gi in pool_groups:
                            scb = scsb.tile([128, SCW], F32, name="scb",
                                            tag="scb")
                            nc.vector.tensor_copy(scb[:, 0:gw],
                                                  sc_ps[:, 0:gw])
                            nc.gpsimd.tensor_tensor(
                                out=ex[:, 0:gw], in0=two[:, 0:gw],
                                in1=scb[:, 0:gw], op=OP.pow)
                        else:
                            nc.scalar.activation(
                                ex[:, 0:gw], sc_ps[:, 0:gw], AF.Exp,
                                bias=(mb[:, tlist[0]:tlist[0] + 1]
                                      if has_mask else 0.0),
                                scale=LN2)
                        if prev is not None:
                            ctx_mms(prev[0], prev[1], last=False)
                        prev = (ex, tlist)
                    ctx_mms(prev[0], prev[1], last=True)
                    work.extend(make_epilogue(h, si, ctx_ps))
            while work:
                work.pop(0)()
            close_qx()
            late.close()


def _build(has_bias, has_mask):
    nc = bacc.Bacc(
        "TRN2", target_bir_lowering=False, debug=False, num_devices=N_CORES
    )
    xt_d = nc.dram_tensor("xt", [D, S], F32R, kind="ExternalInput").ap()
    wq_d = nc.dram_tensor("wq", [D + 1, 128], F32R, kind="ExternalInput").ap()
    wk_d = nc.dram_tensor("wk", [D + 1, VPAD], F32R,
                          kind="ExternalInput").ap()
    wv_d = nc.dram_tensor("wv", [D + 1, VPAD], F32R,
                          kind="ExternalInput").ap()
    id_d = nc.dram_tensor("ident", [128, 128], F32,
                          kind="ExternalInput").ap()
    on_d = nc.dram_tensor("onesd", [128, ST], F32R, kind="ExternalInput").ap()
    mb_d = (
        nc.dram_tensor("mb", [128, NT], F32, kind="ExternalInput").ap()
        if has_mask else None
    )
    out_d = nc.dram_tensor("out", [S, DH], F32, kind="ExternalOutput").ap()

    with tile.TileContext(nc) as tc:
        _emit(tc, (xt_d, wq_d, wk_d, wv_d, id_d, on_d, mb_d, out_d),
              has_bias, has_mask)
    nc.compile()
    return nc


_NC_CACHE = {}


def _get_nc(has_bias, has_mask):
    key = (has_bias, has_mask)
    if key not in _NC_CACHE:
        _NC_CACHE[key] = _build(has_bias, has_mask)
    return _NC_CACHE[key]


def _in_maps(x, Wq, bq, Wk, bk, Wv, bv, mask, has_bias, has_mask):
    ident = np.eye(128, dtype=np.float32)
    xt_by_b = [np.ascontiguousarray(x[b].T) for b in range(B)]
    mb_by_b = [
        np.ascontiguousarray(
            ((np.asarray(mask[b]) == 0).astype(np.float32) * np.float32(-1e30))
            .reshape(NT, 128).T
        )
        for b in range(B)
    ]
    maps = []
    for c in range(N_CORES):
        b, g = divmod(c, N_CORES // B)
        lo = g * DH
        # wq: q heads 0,1 (128 douts), scaled by log2(e)/8
        wq_a = np.empty((D + 1, 128), np.float32)
        wq_a[:D] = Wq[lo:lo + 128, :].T * QSCALE
        wq_a[D] = bq[lo:lo + 128] * QSCALE
        # wk: [k_h01 (128) | k_h2 (64) | q_h2 (64, scaled)]
        wk_a = np.empty((D + 1, VPAD), np.float32)
        wk_a[:D, 0:128] = Wk[lo:lo + 128, :].T
        wk_a[D, 0:128] = bk[lo:lo + 128]
        wk_a[:D, 128:DH] = Wk[lo + 128:lo + DH, :].T
        wk_a[D, 128:DH] = bk[lo + 128:lo + DH]
        wk_a[:D, DH:VPAD] = Wq[lo + 128:lo + DH, :].T * QSCALE
        wk_a[D, DH:VPAD] = bq[lo + 128:lo + DH] * QSCALE
        wv_a = np.zeros((D + 1, VPAD), np.float32)
        wv_a[:D, :DH] = Wv[lo:lo + DH, :].T
        wv_a[D, :DH] = bv[lo:lo + DH]
        m = {
            "xt": _round_f32r(xt_by_b[b]), "wq": _round_f32r(wq_a),
            "wk": _round_f32r(wk_a), "wv": _round_f32r(wv_a), "ident": ident,
            "onesd": np.ones((128, ST), np.float32),
        }
        if has_mask:
            m["mb"] = mb_by_b[b]
        maps.append(m)
    return maps


def _install_ntff_hook():
    """Best-effort: make trace=True work under axon by supplying the
    antenv.axon_hooks shim the boot code degrades without."""
    import types

    try:
        from antenv.axon_hooks import get_axon_ntff_profile_hook  # noqa: F401
        return True
    except ImportError:
        pass
    try:
        import antenv
        from trn_agent_boot.trn_boot import _ntff_profile_via_ctypes

        hook = _ntff_profile_via_ctypes("/opt/axon/libaxon_pjrt.so")
        if hook is None:
            return False
        mod = types.ModuleType("antenv.axon_hooks")
        state = {"hook": hook}
        mod.get_axon_ntff_profile_hook = lambda: state["hook"]
        mod.set_axon_ntff_profile_hook = lambda h: state.update(hook=h)
        sys.modules["antenv.axon_hooks"] = mod
        antenv.axon_hooks = mod
        return True
    except Exception:
        return False


def _run(x, Wq, bq, Wk, bk, Wv, bv, mask, trace=False):
    if trace:
        trace = _install_ntff_hook()
    x = np.ascontiguousarray(np.asarray(x, np.float32))
    Wq = np.asarray(Wq, np.float32)
    Wk = np.asarray(Wk, np.float32)
    Wv = np.asarray(Wv, np.float32)
    bq = np.asarray(bq, np.float32)
    bk = np.asarray(bk, np.float32)
    bv = np.asarray(bv, np.float32)
    has_bias = bool(np.any(bq) or np.any(bk) or np.any(bv))
    has_mask = bool((np.asarray(mask) == 0).any())
    nc = _get_nc(has_bias, has_mask)
    maps = _in_maps(x, Wq, bq, Wk, bk, Wv, bv, mask, has_bias, has_mask)
    res = run_bass_kernel_spmd(nc, maps, list(range(N_CORES)), trace=trace)
    out = np.empty((B, S, D), np.float32)
    for c in range(N_CORES):
        b, g = divmod(c, N_CORES // B)
        out[b, :, g * DH:(g + 1) * DH] = res.results[c]["out"]
    return out, res


def kernel(x, Wq, bq, Wk, bk, Wv, bv, mask):
    out, _ = _run(x, Wq, bq, Wk, bk, Wv, bv, mask)
    return out


# revision 17
# speedup vs baseline: 38.7453x; 1.0243x over previous
"""BERT self-attention (B=2, S=2048, D=768, H=12) on 8 trn2 NeuronCores.

Sharding: core c -> batch b = c//4, head group g = c%4 (3 heads each).
Attention is fully local per core; no collectives.

v6 (all f32 storage; matmuls in float32r fast mode):
  - P0 interleaved per si-block [k(si), v(si), q(si)] so the PE chases the
    x^T DMA stream (input is HBM-bandwidth limited, ~4.4us per si block)
  - psB packing: one matmul produces [k_h2 | q_h2]; q_h2 lands via a
    partition-base-shifted DVE copy
  - P1: pure attention, si-major; exp on ACT in [128,1536] groups; ctx
    accumulates the exp-sum in row 64 (ones column of v_aug); the [65,512]
    ctx^T tiles DMA straight from PSUM to HBM; softmax division and the
    final transpose happen on the host during unsharding
  - ctx PSUM pool is double-buffered so block boundaries never stall the
    ACT exp stream
"""

import sys

import numpy as np

_TRN_REPO = "/opt/trn_rl_repo"
if _TRN_REPO not in sys.path:
    sys.path.insert(0, _TRN_REPO)

import concourse.tile as tile  # noqa: E402
from concourse import bacc, mybir  # noqa: E402
from concourse.bass_utils import run_bass_kernel_spmd  # noqa: E402

F32 = mybir.dt.float32
F32R = mybir.dt.float32r
AF = mybir.ActivationFunctionType

B, S, D = 2, 2048, 768
H_TOT, W = 12, 64
N_CORES = 8
HL = 3                # heads per core
DH = HL * W           # 192 local output dims
KC = D // 128         # 6 contraction chunks of 128
ST = 512              # s-tile (matmul moving free dim)
NS = S // ST          # 4 s-tiles
NT = S // 128         # 16 t-blocks
VPAD = 256            # v-projection free dim padded so float32r runs 1 cyc/row
LN2 = 0.6931471805599453
QSCALE = 0.1803368801111204  # log2(e)/8, folded into Wq on the host


def _round_f32r(a):
    """Round-to-nearest-even fp32 -> fp32r (11-bit mantissa, value kept in
    the top 20 bits of the word) so DMA'd data is already fp32r-valid."""
    u = np.ascontiguousarray(a, np.float32).view(np.uint32).copy()
    u += np.uint32(0x7FF) + ((u >> np.uint32(12)) & np.uint32(1))
    u &= np.uint32(0xFFFFF000)
    return u.view(np.float32)


def _emit(tc, aps, has_bias, has_mask):
    nc = tc.nc
    xt_d, wq_d, wk_d, wv_d, on_d, mb_d, out_d = aps

    from contextlib import ExitStack

    if has_mask:
        groups = [(t,) for t in range(NT)]      # bias varies per t-block
    else:
        groups = [(0, 1, 2), (3, 4, 5), (6, 7, 8), (9, 10, 11),
                  (12, 13, 14), (15,)]
    SCW = len(groups[0]) * ST

    with ExitStack() as ctx:
        const = ctx.enter_context(tc.tile_pool(name="const", bufs=1))

        ones = const.tile([1, ST], F32R, name="ones", tag="ones")
        mb = None
        if has_mask:
            mb = const.tile([128, NT], F32, name="mb", tag="mb")

        # x^T as a single tile [128, KC, S]; weights one tile each.
        # wkall columns: 0:128 = k heads 0,1; 128:192 = k_h2; 192:256 = q_h2.
        xtall = const.tile([128, KC, S], F32R, name="xtall", tag="xtall")
        wkall = const.tile([128, KC, VPAD], F32R, name="wkall", tag="wkall")
        wqall = const.tile([128, KC, 128], F32R, name="wqall", tag="wqall")
        wvall = const.tile([128, KC, VPAD], F32R, name="wvall", tag="wvall")
        vaug = const.tile([128, NT, HL, W + 1], F32R, name="vaug", tag="vaug")
        bias_rows = const.tile([1, 3, VPAD], F32R, name="brows", tag="brows")

        xt = [xtall[:, c, :] for c in range(KC)]
        wk = [wkall[:, c, :] for c in range(KC)]
        wq = [wqall[:, c, :] for c in range(KC)]
        wv = [wvall[:, c, :] for c in range(KC)]
        wqb = bias_rows[:, 0, 0:128]
        wkb = bias_rows[:, 1, :]
        wvb = bias_rows[:, 2, :]

        xt_r = xt_d.rearrange("(c p) s -> p c s", p=128)
        wk_r = wk_d[0:D, :].rearrange("(c p) w -> p c w", p=128)
        wq_r = wq_d[0:D, :].rearrange("(c p) w -> p c w", p=128)
        wv_r = wv_d[0:D, :].rearrange("(c p) w -> p c w", p=128)

        # DMA schedule: the critical first wave (wk + xt si0, split in two)
        # goes out on all three dispatch queues at once; later si blocks and
        # weights stream behind it in the order P0 consumes them.
        nc.scalar.dma_start(out=wkall[:], in_=wk_r)
        nc.sync.dma_start(out=xtall[:, 0:3, 0:ST], in_=xt_r[:, 0:3, 0:ST])
        nc.gpsimd.dma_start(out=xtall[:, 3:6, 0:ST], in_=xt_r[:, 3:6, 0:ST])
        nc.scalar.dma_start(out=wvall[:], in_=wv_r)
        nc.sync.dma_start(out=xtall[:, :, ST:2 * ST],
                          in_=xt_r[:, :, ST:2 * ST])
        nc.scalar.dma_start(out=wqall[:], in_=wq_r)
        nc.gpsimd.dma_start(out=xtall[:, :, 2 * ST:3 * ST],
                            in_=xt_r[:, :, 2 * ST:3 * ST])
        nc.sync.dma_start(out=xtall[:, :, 3 * ST:4 * ST],
                          in_=xt_r[:, :, 3 * ST:4 * ST])
        nc.gpsimd.dma_start(
            out=vaug[:, :, :, W:W + 1],
            in_=on_d[0:128, 0:NT * HL].rearrange("p (t h b) -> p t h b",
                                                 h=HL, b=1))
        nc.scalar.dma_start(out=ones[:], in_=on_d[0:1, :])
        if has_bias:
            nc.sync.dma_start(out=bias_rows[:, 0, 0:128],
                              in_=wq_d[D:D + 1, :])
            nc.sync.dma_start(out=bias_rows[:, 1, :], in_=wk_d[D:D + 1, :])
            nc.sync.dma_start(out=bias_rows[:, 2, :], in_=wv_d[D:D + 1, :])
        if has_mask:
            nc.gpsimd.dma_start(out=mb[:], in_=mb_d[:, :])

        # Projection outputs (persistent). q/k zero-padded on complementary
        # 64 partitions so scores matmuls run K=128.
        qt_h = [const.tile([128, S], F32R, name=f"qt_h{h}", tag=f"qt_h{h}")
                for h in range(HL)]
        kt_a = const.tile([128, S], F32R, name="kt_a", tag="kt_a")
        kt_b = const.tile([128, S], F32R, name="kt_b", tag="kt_b")

        # ---- P0: per si-block [zero-pads, k, v, q] ----------------------
        with tc.tile_pool(name="pA", bufs=2, space="PSUM") as pA, \
             tc.tile_pool(name="pB", bufs=2, space="PSUM") as pB, \
             tc.tile_pool(name="pV", bufs=2, space="PSUM") as pV:

            for si in range(NS):
                ssl = slice(si * ST, (si + 1) * ST)
                # zero the complementary K-padding halves (memset rejects
                # f32r, so multiply freshly-landed x data by 0.0)
                nc.vector.tensor_scalar_mul(qt_h[0][64:128, ssl],
                                            xt[0][0:64, ssl], 0.0)
                nc.vector.tensor_scalar_mul(qt_h[1][0:64, ssl],
                                            xt[0][0:64, ssl], 0.0)
                nc.vector.tensor_scalar_mul(qt_h[2][64:128, ssl],
                                            xt[0][0:64, ssl], 0.0)
                nc.vector.tensor_scalar_mul(kt_b[64:128, ssl],
                                            xt[0][0:64, ssl], 0.0)

                psA = pA.tile([128, ST], F32, name="psA", tag="psA")
                psB = pB.tile([128, ST], F32, name="psB", tag="psB")
                for c in range(KC):
                    nc.tensor.matmul(
                        psA[:], wk[c][:, 0:128], xt[c][:, ssl],
                        start=(c == 0), stop=(c == KC - 1 and not has_bias))
                if has_bias:
                    nc.tensor.matmul(psA[:], wkb[:, 0:128], ones[:],
                                     start=False, stop=True)
                for c in range(KC):
                    nc.tensor.matmul(
                        psB[:], wk[c][:, 128:VPAD], xt[c][:, ssl],
                        start=(c == 0), stop=(c == KC - 1 and not has_bias))
                if has_bias:
                    nc.tensor.matmul(psB[:], wkb[:, 128:VPAD], ones[:],
                                     start=False, stop=True)
                nc.vector.tensor_copy(kt_a[:, ssl], psA[:])
                nc.vector.tensor_copy(kt_b[0:64, ssl], psB[0:64, :])
                # q_h2 rides in psB rows 64:128 -> partition-shifted copy
                nc.vector.tensor_copy(qt_h[2][0:64, ssl], psB[64:128, :])

                for t in range(4 * si, 4 * si + 4):
                    psV = pV.tile([128, VPAD], F32, name="psV", tag="psV")
                    tsl = slice(t * 128, (t + 1) * 128)
                    for c in range(KC):
                        nc.tensor.matmul(
                            psV[:], xt[c][:, tsl], wv[c][:],
                            start=(c == 0),
                            stop=(c == KC - 1 and not has_bias))
                    if has_bias:
                        nc.tensor.matmul(psV[:], ones[:, 0:128], wvb[:],
                                         start=False, stop=True)
                    nc.vector.tensor_copy(
                        vaug[:, t, :, 0:W],
                        psV[:, 0:DH].rearrange("p (h w) -> p h w", h=HL))

                psQ = pA.tile([128, ST], F32, name="psA", tag="psA")
                for c in range(KC):
                    nc.tensor.matmul(
                        psQ[:], wq[c][:], xt[c][:, ssl],
                        start=(c == 0), stop=(c == KC - 1 and not has_bias))
                if has_bias:
                    nc.tensor.matmul(psQ[:], wqb[:], ones[:],
                                     start=False, stop=True)
                nc.vector.tensor_copy(qt_h[0][0:64, ssl], psQ[0:64, :])
                nc.vector.tensor_copy(qt_h[1][64:128, ssl], psQ[64:128, :])

        # ---- P1: pure attention, si-major -------------------------------
        with tc.tile_pool(name="scps", bufs=2, space="PSUM") as scp, \
             tc.tile_pool(name="ctxps", bufs=2, space="PSUM") as cxp, \
             tc.tile_pool(name="expool", bufs=3) as exp_pool, \
             tc.tile_pool(name="ctxsb", bufs=2) as csb:

            for si in range(NS):
                for h in range(HL):
                    ktile = kt_a if h < 2 else kt_b
                    qtile = qt_h[h]
                    ssl = slice(si * ST, (si + 1) * ST)
                    ctx_ps = cxp.tile([65, ST], F32, name="ctx_ps",
                                      tag="ctx_ps")
                    prev = None
                    n_acc = [0]

                    def ctx_mms(ex, tlist, last):
                        for j, t in enumerate(tlist):
                            nc.tensor.matmul(
                                ctx_ps[:], vaug[:, t, h, :],
                                ex[:, j * ST:(j + 1) * ST],
                                start=(n_acc[0] == 0),
                                stop=(last and j == len(tlist) - 1))
                            n_acc[0] += 1

                    for gi, tlist in enumerate(groups):
                        gw = len(tlist) * ST
                        sc_ps = scp.tile([128, SCW], F32, name="sc_ps",
                                         tag="sc_ps")
                        for j, t in enumerate(tlist):
                            nc.tensor.matmul(
                                sc_ps[:, j * ST:(j + 1) * ST],
                                ktile[:, t * 128:(t + 1) * 128],
                                qtile[:, ssl], start=True, stop=True)
                        ex = exp_pool.tile([128, SCW], F32R, name="ex",
                                           tag="ex")
                        nc.scalar.activation(
                            ex[:, 0:gw], sc_ps[:, 0:gw], AF.Exp,
                            bias=(mb[:, tlist[0]:tlist[0] + 1]
                                  if has_mask else 0.0),
                            scale=LN2)
                        if prev is not None:
                            ctx_mms(prev[0], prev[1], last=False)
                        prev = (ex, tlist)
                    ctx_mms(prev[0], prev[1], last=True)
                    # ship ctx^T (incl. exp-sums in row 64) to HBM via a DVE
                    # bounce to SBUF (DMA cannot read PSUM); host divides +
                    # transposes during unsharding
                    ctx_sb = csb.tile([65, ST], F32, name="ctx_sb",
                                      tag="ctx_sb")
                    nc.vector.tensor_copy(ctx_sb[:], ctx_ps[:])
                    nc.sync.dma_start(out=out_d[h, si, :, :], in_=ctx_sb[:])


def _build(has_bias, has_mask):
    nc = bacc.Bacc(
        "TRN2", target_bir_lowering=False, debug=False, num_devices=N_CORES
    )
    xt_d = nc.dram_tensor("xt", [D, S], F32R, kind="ExternalInput").ap()
    wq_d = nc.dram_tensor("wq", [D + 1, 128], F32R, kind="ExternalInput").ap()
    wk_d = nc.dram_tensor("wk", [D + 1, VPAD], F32R,
                          kind="ExternalInput").ap()
    wv_d = nc.dram_tensor("wv", [D + 1, VPAD], F32R,
                          kind="ExternalInput").ap()
    on_d = nc.dram_tensor("onesd", [128, ST], F32R, kind="ExternalInput").ap()
    mb_d = (
        nc.dram_tensor("mb", [128, NT], F32, kind="ExternalInput").ap()
        if has_mask else None
    )
    out_d = nc.dram_tensor("out", [HL, NS, W + 1, ST], F32,
                           kind="ExternalOutput").ap()

    with tile.TileContext(nc) as tc:
        _emit(tc, (xt_d, wq_d, wk_d, wv_d, on_d, mb_d, out_d),
              has_bias, has_mask)
    nc.compile()
    return nc


_NC_CACHE = {}


def _get_nc(has_bias, has_mask):
    key = (has_bias, has_mask)
    if key not in _NC_CACHE:
        _NC_CACHE[key] = _build(has_bias, has_mask)
    return _NC_CACHE[key]


def _in_maps(x, Wq, bq, Wk, bk, Wv, bv, mask, has_bias, has_mask):
    xt_by_b = [np.ascontiguousarray(x[b].T) for b in range(B)]
    mb_by_b = [
        np.ascontiguousarray(
            ((np.asarray(mask[b]) == 0).astype(np.float32) * np.float32(-1e30))
            .reshape(NT, 128).T
        )
        for b in range(B)
    ]
    maps = []
    for c in range(N_CORES):
        b, g = divmod(c, N_CORES // B)
        lo = g * DH
        # wq: q heads 0,1 (128 douts), scaled by log2(e)/8
        wq_a = np.empty((D + 1, 128), np.float32)
        wq_a[:D] = Wq[lo:lo + 128, :].T * QSCALE
        wq_a[D] = bq[lo:lo + 128] * QSCALE
        # wk: [k_h01 (128) | k_h2 (64) | q_h2 (64, scaled)]
        wk_a = np.empty((D + 1, VPAD), np.float32)
        wk_a[:D, 0:128] = Wk[lo:lo + 128, :].T
        wk_a[D, 0:128] = bk[lo:lo + 128]
        wk_a[:D, 128:DH] = Wk[lo + 128:lo + DH, :].T
        wk_a[D, 128:DH] = bk[lo + 128:lo + DH]
        wk_a[:D, DH:VPAD] = Wq[lo + 128:lo + DH, :].T * QSCALE
        wk_a[D, DH:VPAD] = bq[lo + 128:lo + DH] * QSCALE
        wv_a = np.zeros((D + 1, VPAD), np.float32)
        wv_a[:D, :DH] = Wv[lo:lo + DH, :].T
        wv_a[D, :DH] = bv[lo:lo + DH]
        m = {
            "xt": _round_f32r(xt_by_b[b]), "wq": _round_f32r(wq_a),
            "wk": _round_f32r(wk_a), "wv": _round_f32r(wv_a),
            "onesd": np.ones((128, ST), np.float32),
        }
        if has_mask:
            m["mb"] = mb_by_b[b]
        maps.append(m)
    return maps


def _install_ntff_hook():
    """Best-effort: make trace=True work under axon by supplying the
    antenv.axon_hooks shim the boot code degrades without."""
    import types

    try:
        from antenv.axon_hooks import get_axon_ntff_profile_hook  # noqa: F401
        return True
    except ImportError:
        pass
    try:
        import antenv
        from trn_agent_boot.trn_boot import _ntff_profile_via_ctypes

        hook = _ntff_profile_via_ctypes("/opt/axon/libaxon_pjrt.so")
        if hook is None:
            return False
        mod = types.ModuleType("antenv.axon_hooks")
        state = {"hook": hook}
        mod.get_axon_ntff_profile_hook = lambda: state["hook"]
        mod.set_axon_ntff_profile_hook = lambda h: state.update(hook=h)
        sys.modules["antenv.axon_hooks"] = mod
        antenv.axon_hooks = mod
        return True
    except Exception:
        return False


def _run(x, Wq, bq, Wk, bk, Wv, bv, mask, trace=False):
    if trace:
        trace = _install_ntff_hook()
    x = np.ascontiguousarray(np.asarray(x, np.float32))
    Wq = np.asarray(Wq, np.float32)
    Wk = np.asarray(Wk, np.float32)
    Wv = np.asarray(Wv, np.float32)
    bq = np.asarray(bq, np.float32)
    bk = np.asarray(bk, np.float32)
    bv = np.asarray(bv, np.float32)
    has_bias = bool(np.any(bq) or np.any(bk) or np.any(bv))
    has_mask = bool((np.asarray(mask) == 0).any())
    nc = _get_nc(has_bias, has_mask)
    maps = _in_maps(x, Wq, bq, Wk, bk, Wv, bv, mask, has_bias, has_mask)
    res = run_bass_kernel_spmd(nc, maps, list(range(N_CORES)), trace=trace)
    out = np.empty((B, S, D), np.float32)
    for c in range(N_CORES):
        b, g = divmod(c, N_CORES // B)
        ct = np.asarray(res.results[c]["out"], np.float64)  # [HL,NS,65,ST]
        for h in range(HL):
            num = ct[h, :, 0:W, :]          # [NS, W, ST]
            den = ct[h, :, W:W + 1, :]      # [NS, 1, ST]
            o = (num / den).transpose(0, 2, 1).reshape(S, W)
            out[b, :, g * DH + h * W:(g * DH) + (h + 1) * W] = o
    return out, res


def kernel(x, Wq, bq, Wk, bk, Wv, bv, mask):
    out, _ = _run(x, Wq, bq, Wk, bk, Wv, bv, mask)
    return out


# revision 23
# speedup vs baseline: 44.6801x; 1.1532x over previous
"""BERT self-attention (B=2, S=2048, D=768, H=12) on 8 trn2 NeuronCores.

Sharding: core c -> batch b = c//4, head group g = c%4 (3 heads each).
Attention is fully local per core; no collectives.

v6 (all f32 storage; matmuls in float32r fast mode):
  - P0 interleaved per si-block [k(si), v(si), q(si)] so the PE chases the
    x^T DMA stream (input is HBM-bandwidth limited, ~4.4us per si block)
  - psB packing: one matmul produces [k_h2 | q_h2]; q_h2 lands via a
    partition-base-shifted DVE copy
  - P1: pure attention, si-major; exp on ACT in [128,1536] groups; ctx
    accumulates the exp-sum in row 64 (ones column of v_aug); the [65,512]
    ctx^T tiles DMA straight from PSUM to HBM; softmax division and the
    final transpose happen on the host during unsharding
  - ctx PSUM pool is double-buffered so block boundaries never stall the
    ACT exp stream
"""

import sys

import numpy as np

_TRN_REPO = "/opt/trn_rl_repo"
if _TRN_REPO not in sys.path:
    sys.path.insert(0, _TRN_REPO)

import concourse.tile as tile  # noqa: E402
from concourse import bacc, mybir  # noqa: E402
from concourse.bass_utils import run_bass_kernel_spmd  # noqa: E402

F32 = mybir.dt.float32
F32R = mybir.dt.float32r
AF = mybir.ActivationFunctionType

B, S, D = 2, 2048, 768
H_TOT, W = 12, 64
N_CORES = 8
HL = 3                # heads per core
DH = HL * W           # 192 local output dims
KC = D // 128         # 6 contraction chunks of 128
ST = 512              # s-tile (matmul moving free dim)
NS = S // ST          # 4 s-tiles
NT = S // 128         # 16 t-blocks
VPAD = 256            # v-projection free dim padded so float32r runs 1 cyc/row
LN2 = 0.6931471805599453
QSCALE = 0.1803368801111204  # log2(e)/8, folded into Wq on the host


def _round_f32r(a):
    """Round-to-nearest-even fp32 -> fp32r (11-bit mantissa, value kept in
    the top 20 bits of the word) so DMA'd data is already fp32r-valid."""
    u = np.ascontiguousarray(a, np.float32).view(np.uint32).copy()
    u += np.uint32(0x7FF) + ((u >> np.uint32(12)) & np.uint32(1))
    u &= np.uint32(0xFFFFF000)
    return u.view(np.float32)


def _emit(tc, aps, has_bias, has_mask):
    nc = tc.nc
    xt_d, wq_d, wk_d, wv_d, on_d, mb_d, out_d = aps

    from contextlib import ExitStack

    if has_mask:
        groups = [(t,) for t in range(NT)]      # bias varies per t-block
    else:
        groups = [(0, 1, 2), (3, 4, 5), (6, 7, 8), (9, 10, 11),
                  (12, 13, 14), (15,)]
    SCW = len(groups[0]) * ST

    with ExitStack() as ctx:
        const = ctx.enter_context(tc.tile_pool(name="const", bufs=1))

        ones = const.tile([1, ST], F32R, name="ones", tag="ones")
        mb = None
        if has_mask:
            mb = const.tile([128, NT], F32, name="mb", tag="mb")

        # x^T as a single tile [128, KC, S]; weights one tile each.
        # wkall columns: 0:128 = k heads 0,1; 128:192 = k_h2; 192:256 = q_h2.
        xtall = const.tile([128, KC, S], F32R, name="xtall", tag="xtall")
        wkall = const.tile([128, KC, VPAD], F32R, name="wkall", tag="wkall")
        wqall = const.tile([128, KC, 128], F32R, name="wqall", tag="wqall")
        wvall = const.tile([128, KC, VPAD], F32R, name="wvall", tag="wvall")
        vaug = const.tile([128, NT, HL, W + 1], F32R, name="vaug", tag="vaug")
        bias_rows = const.tile([1, 3, VPAD], F32R, name="brows", tag="brows")

        stamp = const.tile([1, NS], F32, name="stamp", tag="stamp")
        stamp2 = const.tile([1, NS], F32, name="stamp2", tag="stamp2")

        xt = [xtall[:, c, :] for c in range(KC)]
        wk = [wkall[:, c, :] for c in range(KC)]
        wq = [wqall[:, c, :] for c in range(KC)]
        wv = [wvall[:, c, :] for c in range(KC)]
        wqb = bias_rows[:, 0, 0:128]
        wkb = bias_rows[:, 1, :]
        wvb = bias_rows[:, 2, :]

        xt_r = xt_d.rearrange("(c p) s -> p c s", p=128)
        wk_r = wk_d[0:D, :].rearrange("(c p) w -> p c w", p=128)
        wq_r = wq_d[0:D, :].rearrange("(c p) w -> p c w", p=128)
        wv_r = wv_d[0:D, :].rearrange("(c p) w -> p c w", p=128)

        # DMA schedule: only the critical first wave (wk+wv+wq + xt si0)
        # goes out at program start; the later xt si-blocks are dispatched
        # from the DVE's instruction stream inside P0, so they do not steal
        # HBM bandwidth from the wave the PE is waiting on.
        nc.sync.dma_start(out=xtall[:, 0:3, 0:ST], in_=xt_r[:, 0:3, 0:ST])
        nc.gpsimd.dma_start(out=xtall[:, 3:6, 0:ST], in_=xt_r[:, 3:6, 0:ST])
        nc.scalar.dma_start(out=wkall[:], in_=wk_r)
        nc.scalar.dma_start(out=wvall[:], in_=wv_r)
        nc.scalar.dma_start(out=wqall[:], in_=wq_r)
        nc.gpsimd.dma_start(
            out=vaug[:, :, :, W:W + 1],
            in_=on_d[0:128, 0:NT * HL].rearrange("p (t h b) -> p t h b",
                                                 h=HL, b=1))
        nc.scalar.dma_start(out=ones[:], in_=on_d[0:1, :])
        if has_bias:
            nc.sync.dma_start(out=bias_rows[:, 0, 0:128],
                              in_=wq_d[D:D + 1, :])
            nc.sync.dma_start(out=bias_rows[:, 1, :], in_=wk_d[D:D + 1, :])
            nc.sync.dma_start(out=bias_rows[:, 2, :], in_=wv_d[D:D + 1, :])
        if has_mask:
            nc.gpsimd.dma_start(out=mb[:], in_=mb_d[:, :])

        # Projection outputs (persistent). q/k zero-padded on complementary
        # 64 partitions so scores matmuls run K=128.
        qt_h = [const.tile([128, S], F32R, name=f"qt_h{h}", tag=f"qt_h{h}")
                for h in range(HL)]
        kt_a = const.tile([128, S], F32R, name="kt_a", tag="kt_a")
        kt_b = const.tile([128, S], F32R, name="kt_b", tag="kt_b")

        # ---- P0: per si-block [zero-pads, k, v, q] ----------------------
        with tc.tile_pool(name="pA", bufs=2, space="PSUM") as pA, \
             tc.tile_pool(name="pB", bufs=2, space="PSUM") as pB, \
             tc.tile_pool(name="pV", bufs=2, space="PSUM") as pV:

            for si in range(NS):
                ssl = slice(si * ST, (si + 1) * ST)
                # zero the complementary K-padding halves (memset rejects
                # f32r, so multiply freshly-landed x data by 0.0)
                nc.vector.tensor_scalar_mul(qt_h[0][64:128, ssl],
                                            xt[0][0:64, ssl], 0.0)
                # gate the next si block's DMA on this block's x having
                # landed: DVE writes a stamp (in-order after the mul above),
                # gpsimd waits on it and only then dispatches the DMA, so
                # later waves don't steal HBM bandwidth from the current one
                if si + 1 < NS:
                    nsl = slice((si + 1) * ST, (si + 2) * ST)
                    nc.vector.tensor_scalar_mul(stamp[0:1, si:si + 1],
                                                ones[0:1, 0:1], 0.0)
                    nc.gpsimd.tensor_tensor(
                        out=stamp2[0:1, si:si + 1],
                        in0=stamp[0:1, si:si + 1],
                        in1=stamp[0:1, si:si + 1],
                        op=mybir.AluOpType.mult)
                    nc.gpsimd.dma_start(out=xtall[:, :, nsl],
                                        in_=xt_r[:, :, nsl])
                nc.vector.tensor_scalar_mul(qt_h[1][0:64, ssl],
                                            xt[0][0:64, ssl], 0.0)
                nc.vector.tensor_scalar_mul(qt_h[2][64:128, ssl],
                                            xt[0][0:64, ssl], 0.0)
                nc.vector.tensor_scalar_mul(kt_b[64:128, ssl],
                                            xt[0][0:64, ssl], 0.0)

                psA = pA.tile([128, ST], F32, name="psA", tag="psA")
                psB = pB.tile([128, ST], F32, name="psB", tag="psB")
                for c in range(KC):
                    nc.tensor.matmul(
                        psA[:], wk[c][:, 0:128], xt[c][:, ssl],
                        start=(c == 0), stop=(c == KC - 1 and not has_bias))
                if has_bias:
                    nc.tensor.matmul(psA[:], wkb[:, 0:128], ones[:],
                                     start=False, stop=True)
                for c in range(KC):
                    nc.tensor.matmul(
                        psB[:], wk[c][:, 128:VPAD], xt[c][:, ssl],
                        start=(c == 0), stop=(c == KC - 1 and not has_bias))
                if has_bias:
                    nc.tensor.matmul(psB[:], wkb[:, 128:VPAD], ones[:],
                                     start=False, stop=True)
                nc.vector.tensor_copy(kt_a[:, ssl], psA[:])
                nc.vector.tensor_copy(kt_b[0:64, ssl], psB[0:64, :])
                # q_h2 rides in psB rows 64:128 -> partition-shifted copy
                nc.vector.tensor_copy(qt_h[2][0:64, ssl], psB[64:128, :])

                for t in range(4 * si, 4 * si + 4):
                    psV = pV.tile([128, VPAD], F32, name="psV", tag="psV")
                    tsl = slice(t * 128, (t + 1) * 128)
                    for c in range(KC):
                        nc.tensor.matmul(
                            psV[:], xt[c][:, tsl], wv[c][:],
                            start=(c == 0),
                            stop=(c == KC - 1 and not has_bias))
                    if has_bias:
                        nc.tensor.matmul(psV[:], ones[:, 0:128], wvb[:],
                                         start=False, stop=True)
                    nc.vector.tensor_copy(
                        vaug[:, t, :, 0:W],
                        psV[:, 0:DH].rearrange("p (h w) -> p h w", h=HL))

                psQ = pA.tile([128, ST], F32, name="psA", tag="psA")
                for c in range(KC):
                    nc.tensor.matmul(
                        psQ[:], wq[c][:], xt[c][:, ssl],
                        start=(c == 0), stop=(c == KC - 1 and not has_bias))
                if has_bias:
                    nc.tensor.matmul(psQ[:], wqb[:], ones[:],
                                     start=False, stop=True)
                nc.vector.tensor_copy(qt_h[0][0:64, ssl], psQ[0:64, :])
                nc.vector.tensor_copy(qt_h[1][64:128, ssl], psQ[64:128, :])

        # ---- P1: pure attention, si-major -------------------------------
        with tc.tile_pool(name="scps", bufs=2, space="PSUM") as scp, \
             tc.tile_pool(name="ctxps", bufs=2, space="PSUM") as cxp, \
             tc.tile_pool(name="expool", bufs=4) as exp_pool, \
             tc.tile_pool(name="ctxsb", bufs=2) as csb:

            # Flat stream over (si, h) blocks with a 2-group-lagged ctx
            # FIFO that crosses block boundaries: each block's next scores
            # are emitted before the previous block's trailing ctx groups,
            # so the ACT exp stream never waits for the PE at boundaries.
            fifo = []

            def make_ctx_item(ctx_ps, n_acc, ex, tlist, last, h, si):
                def run():
                    for j, t in enumerate(tlist):
                        nc.tensor.matmul(
                            ctx_ps[:], vaug[:, t, h, :],
                            ex[:, j * ST:(j + 1) * ST],
                            start=(n_acc[0] == 0),
                            stop=(last and j == len(tlist) - 1))
                        n_acc[0] += 1
                    if last:
                        # ship ctx^T (incl. exp-sums in row 64) to HBM via a
                        # DVE bounce to SBUF (DMA cannot read PSUM); host
                        # divides + transposes during unsharding
                        ctx_sb = csb.tile([65, ST], F32, name="ctx_sb",
                                          tag="ctx_sb")
                        nc.vector.tensor_copy(ctx_sb[:], ctx_ps[:])
                        nc.sync.dma_start(out=out_d[h, si, :, :],
                                          in_=ctx_sb[:])
                return run

            for si in range(NS):
                for h in range(HL):
                    ktile = kt_a if h < 2 else kt_b
                    qtile = qt_h[h]
                    ssl = slice(si * ST, (si + 1) * ST)
                    ctx_ps = cxp.tile([65, ST], F32, name="ctx_ps",
                                      tag="ctx_ps")
                    n_acc = [0]
                    for gi, tlist in enumerate(groups):
                        gw = len(tlist) * ST
                        sc_ps = scp.tile([128, SCW], F32, name="sc_ps",
                                         tag="sc_ps")
                        for j, t in enumerate(tlist):
                            nc.tensor.matmul(
                                sc_ps[:, j * ST:(j + 1) * ST],
                                ktile[:, t * 128:(t + 1) * 128],
                                qtile[:, ssl], start=True, stop=True)
                        while len(fifo) >= 2:
                            fifo.pop(0)()
                        ex = exp_pool.tile([128, SCW], F32R, name="ex",
                                           tag="ex")
                        nc.scalar.activation(
                            ex[:, 0:gw], sc_ps[:, 0:gw], AF.Exp,
                            bias=(mb[:, tlist[0]:tlist[0] + 1]
                                  if has_mask else 0.0),
                            scale=LN2)
                        fifo.append(make_ctx_item(
                            ctx_ps, n_acc, ex, tlist,
                            gi == len(groups) - 1, h, si))
            while fifo:
                fifo.pop(0)()


def _build(has_bias, has_mask):
    nc = bacc.Bacc(
        "TRN2", target_bir_lowering=False, debug=False, num_devices=N_CORES
    )
    xt_d = nc.dram_tensor("xt", [D, S], F32R, kind="ExternalInput").ap()
    wq_d = nc.dram_tensor("wq", [D + 1, 128], F32R, kind="ExternalInput").ap()
    wk_d = nc.dram_tensor("wk", [D + 1, VPAD], F32R,
                          kind="ExternalInput").ap()
    wv_d = nc.dram_tensor("wv", [D + 1, VPAD], F32R,
                          kind="ExternalInput").ap()
    on_d = nc.dram_tensor("onesd", [128, ST], F32R, kind="ExternalInput").ap()
    mb_d = (
        nc.dram_tensor("mb", [128, NT], F32, kind="ExternalInput").ap()
        if has_mask else None
    )
    out_d = nc.dram_tensor("out", [HL, NS, W + 1, ST], F32,
                           kind="ExternalOutput").ap()

    with tile.TileContext(nc) as tc:
        _emit(tc, (xt_d, wq_d, wk_d, wv_d, on_d, mb_d, out_d),
              has_bias, has_mask)
    nc.compile()
    return nc


_NC_CACHE = {}


def _get_nc(has_bias, has_mask):
    key = (has_bias, has_mask)
    if key not in _NC_CACHE:
        _NC_CACHE[key] = _build(has_bias, has_mask)
    return _NC_CACHE[key]


def _in_maps(x, Wq, bq, Wk, bk, Wv, bv, mask, has_bias, has_mask):
    xt_by_b = [np.ascontiguousarray(x[b].T) for b in range(B)]
    mb_by_b = [
        np.ascontiguousarray(
            ((np.asarray(mask[b]) == 0).astype(np.float32) * np.float32(-1e30))
            .reshape(NT, 128).T
        )
        for b in range(B)
    ]
    maps = []
    for c in range(N_CORES):
        b, g = divmod(c, N_CORES // B)
        lo = g * DH
        # wq: q heads 0,1 (128 douts), scaled by log2(e)/8
        wq_a = np.empty((D + 1, 128), np.float32)
        wq_a[:D] = Wq[lo:lo + 128, :].T * QSCALE
        wq_a[D] = bq[lo:lo + 128] * QSCALE
        # wk: [k_h01 (128) | k_h2 (64) | q_h2 (64, scaled)]
        wk_a = np.empty((D + 1, VPAD), np.float32)
        wk_a[:D, 0:128] = Wk[lo:lo + 128, :].T
        wk_a[D, 0:128] = bk[lo:lo + 128]
        wk_a[:D, 128:DH] = Wk[lo + 128:lo + DH, :].T
        wk_a[D, 128:DH] = bk[lo + 128:lo + DH]
        wk_a[:D, DH:VPAD] = Wq[lo + 128:lo + DH, :].T * QSCALE
        wk_a[D, DH:VPAD] = bq[lo + 128:lo + DH] * QSCALE
        wv_a = np.zeros((D + 1, VPAD), np.float32)
        wv_a[:D, :DH] = Wv[lo:lo + DH, :].T
        wv_a[D, :DH] = bv[lo:lo + DH]
        m = {
            "xt": _round_f32r(xt_by_b[b]), "wq": _round_f32r(wq_a),
            "wk": _round_f32r(wk_a), "wv": _round_f32r(wv_a),
            "onesd": np.ones((128, ST), np.float32),
        }
        if has_mask:
            m["mb"] = mb_by_b[b]
        maps.append(m)
    return maps


def _install_ntff_hook():
    """Best-effort: make trace=True work under axon by supplying the
    antenv.axon_hooks shim the boot code degrades without."""
    import types

    try:
        from antenv.axon_hooks import get_axon_ntff_profile_hook  # noqa: F401
        return True
    except ImportError:
        pass
    try:
        import antenv
        from trn_agent_boot.trn_boot import _ntff_profile_via_ctypes

        hook = _ntff_profile_via_ctypes("/opt/axon/libaxon_pjrt.so")
        if hook is None:
            return False
        mod = types.ModuleType("antenv.axon_hooks")
        state = {"hook": hook}
        mod.get_axon_ntff_profile_hook = lambda: state["hook"]
        mod.set_axon_ntff_profile_hook = lambda h: state.update(hook=h)
        sys.modules["antenv.axon_hooks"] = mod
        antenv.axon_hooks = mod
        return True
    except Exception:
        return False


def _run(x, Wq, bq, Wk, bk, Wv, bv, mask, trace=False):
    if trace:
        trace = _install_ntff_hook()
    x = np.ascontiguousarray(np.asarray(x, np.float32))
    Wq = np.asarray(Wq, np.float32)
    Wk = np.asarray(Wk, np.float32)
    Wv = np.asarray(Wv, np.float32)
    bq = np.asarray(bq, np.float32)
    bk = np.asarray(bk, np.float32)
    bv = np.asarray(bv, np.float32)
    has_bias = bool(np.any(bq) or np.any(bk) or np.any(bv))
    has_mask = bool((np.asarray(mask) == 0).any())
    nc = _get_nc(has_bias, has_mask)
    maps = _in_maps(x, Wq, bq, Wk, bk, Wv, bv, mask, has_bias, has_mask)
    res = run_bass_kernel_spmd(nc, maps, list(range(N_CORES)), trace=trace)
    out = np.empty((B, S, D), np.float32)
    for c in range(N_CORES):
        b, g = divmod(c, N_CORES // B)
        ct = np.asarray(res.results[c]["out"], np.float64)  # [HL,NS,65,ST]
        for h in range(HL):
            num = ct[h, :, 0:W, :]          # [NS, W, ST]
            den = ct[h, :, W:W + 1, :]      # [NS, 1, ST]
            o = (num / den).transpose(0, 2, 1).reshape(S, W)
            out[b, :, g * DH + h * W:(g * DH) + (h + 1) * W] = o
    return out, res


def kernel(x, Wq, bq, Wk, bk, Wv, bv, mask):
    out, _ = _run(x, Wq, bq, Wk, bk, Wv, bv, mask)
    return out
